# revision 20
# baseline (speedup 1.0000x reference)
"""TRN2 Bass kernel for nn_CausalSelfAttention_4054449128214.

The reference returns out_s + stop_gradient(out_full - out_s), whose forward
value is exactly out_full — plain dense causal self-attention. So the kernel
computes: qkv = x@W_attn+b_attn, per-head causal softmax attention, y@W_proj+b_proj.

Sharding (8 cores, no collectives):
  Megatron head-parallel. Cores 0-3 own head pairs (0,1)..(6,7); cores 4-7 own
  heads 8..11 (run twice for SPMD shape-uniformity, second copy's W_proj rows
  zeroed). Each core computes its heads' Q/K columns, V^T directly via matmul
  (lhsT = x chunk, rhs = W_v), attention, and a partial row-sliced output
  projection; the host sums the 8 partials (the Megatron row-parallel
  all-reduce) and transposes back.

Perf structure:
  - one consumption-ordered dram blob; per-cc segments (wqk|wv|x0) so each DMA
    unlocks a full contraction chunk of K/Q/V^T work (HWDGE issue is a serial
    625ns/DMA resource).
  - all matmul operands bf16 (full PE rate, half DMA bytes); fp32 PSUM.
  - PE p-state warmup: tiny matmuls during the initial DMA wait.
  - K/Q/V^T accumulation groups interleaved per-cc; attention interleaved
    with the next phase's matmuls.
  - attention query tiles split into 256-wide halves A/B with separate PSUM
    accumulators: causality lets half A skip the last two key chunks, so the
    A-half softmax/proj/store complete while B's chunks still run; the kernel
    tail is only the 256-wide B chain + 12 small proj matmuls + a tiny store.
  - bias algebra: b_k is softmax-invariant (dropped); b_v/b_proj fold into a
    host-side constant (softmax rows sum to 1); only b_q in-kernel.
"""

import numpy as np
import ml_dtypes

import concourse.bacc as bacc
import concourse.mybir as mybir
import concourse.tile as tile
from concourse.bass_utils import run_bass_kernel_spmd

F32 = mybir.dt.float32
BF16 = mybir.dt.bfloat16

T = 1024          # sequence length
C = 768           # channels
NH = 12           # heads
HS = 64           # head size
NCORES = 8
TT = 512          # t-tile (query tile)
H = TT // 2       # query half-tile
NT = T // TT      # 2
NCC = C // 128    # 6 contraction chunks
NKC = T // 128    # 8 key chunks
SCALE = 1.0 / 8.0  # 1/sqrt(HS)

NDUMMY = 200              # PE p-state warmup matmuls (run during initial DMA wait)
SEG = 896                 # per-cc blob segment: wqk(256) | wv(128) | x0(512)
X1OFF = NCC * SEG         # 5376
WPOFF = X1OFF + NCC * TT  # 8448
BLOBW = WPOFF + C         # 9216

# core -> (head0, head1); cores 4-7 duplicate their head (2nd W_proj slice zeroed)
HEAD_MAP = [(0, 1), (2, 3), (4, 5), (6, 7), (8, 8), (9, 9), (10, 10), (11, 11)]

_CACHE: dict = {}


def _build_program():
    nc = bacc.Bacc("TRN2", target_bir_lowering=False, debug=False,
                   num_devices=NCORES)
    blob = nc.dram_tensor("blob", [128, BLOBW], BF16, kind="ExternalInput").ap()
    bq = nc.dram_tensor("bq", [128, 1], F32, kind="ExternalInput").ap()
    outT = nc.dram_tensor("outT", [C, T], BF16, kind="ExternalOutput").ap()

    with tile.TileContext(nc) as tc:
        with (
            tc.tile_pool(name="const", bufs=1) as cp,
            tc.tile_pool(name="e", bufs=12) as ep,
            tc.tile_pool(name="rb", bufs=6) as rbp,
            tc.tile_pool(name="pmm", bufs=2, space="PSUM") as pmm,
            tc.tile_pool(name="pst", bufs=2, space="PSUM") as pst,
            tc.tile_pool(name="pov", bufs=2, space="PSUM") as pov,
            tc.tile_pool(name="pvt", bufs=2, space="PSUM") as pvt,
        ):
            bsb = cp.tile([128, WPOFF], BF16, tag="bsb")
            wpt = cp.tile([128, C], BF16, tag="wpt")
            for cc in range(NCC):
                nc.sync.dma_start(out=bsb[:, cc * SEG:(cc + 1) * SEG],
                                  in_=blob[:, cc * SEG:(cc + 1) * SEG])
            nc.sync.dma_start(out=bsb[:, X1OFF:X1OFF + 3 * TT],
                              in_=blob[:, X1OFF:X1OFF + 3 * TT])
            nc.sync.dma_start(out=bsb[:, X1OFF + 3 * TT:WPOFF],
                              in_=blob[:, X1OFF + 3 * TT:WPOFF])
            nc.sync.dma_start(out=wpt[:], in_=blob[:, WPOFF:BLOBW])
            bq_sb = cp.tile([128, 1], F32, tag="bq")
            nc.gpsimd.dma_start(out=bq_sb[:], in_=bq)

            wqk = [bsb[:, cc * SEG:cc * SEG + 256] for cc in range(NCC)]
            wv = [bsb[:, cc * SEG + 256:cc * SEG + 384] for cc in range(NCC)]
            xts = [[bsb[:, cc * SEG + 384:(cc + 1) * SEG],
                    bsb[:, X1OFF + cc * TT:X1OFF + (cc + 1) * TT]]
                   for cc in range(NCC)]

            ones64 = cp.tile([1, 64], BF16, tag="ones64")
            nc.vector.memset(ones64[:], 1.0)
            # PE warmup: tiny matmuls during the initial DMA wait keep the
            # tensor engine's p-state ramping so real work runs at full clock
            pdum = pvt.tile([1, 1], F32, tag="pt", name="pdum")
            for _ in range(NDUMMY):
                nc.tensor.matmul(pdum[:], ones64[0:1, 0:1], ones64[0:1, 0:1],
                                 start=True, stop=True)

            # V^T tiles: [128 keys, 65*2] with a ones column at 64 and 129
            vaug = [cp.tile([128, 130], BF16, tag=f"va{kc}", name=f"va{kc}")
                    for kc in range(NKC)]
            for kc in range(NKC):
                nc.vector.memset(vaug[kc][:, 64:130:65], 1.0)

            # static causal mask (keep where query-in-block >= key-in-block)
            # for the DVE half of the diagonal-mask work; with shifted tiles
            # the diagonal is always the first 128 columns of the tile
            mask0 = cp.tile([128, 128], BF16, tag="mask0")
            nc.vector.memset(mask0[:], 1.0)
            nc.gpsimd.affine_select(
                mask0[:], mask0[:], pattern=[[1, 128]],
                compare_op=mybir.AluOpType.is_ge, fill=0.0,
                base=0, channel_multiplier=-1)

            qT = [None] * NT   # [128=(2h x 64hs), TT] bf16
            kT = [None] * NT
            yT = [None] * NT   # [128=(2h x 64hs), TT] bf16 (both heads stacked)
            st8 = {}           # per-tt K/Q/V01 psum state

            def vcopy(kc, pt):
                nc.vector.tensor_copy(vaug[kc][:, 0:64], pt[:, 0:64])
                nc.vector.tensor_copy(vaug[kc][:, 65:129], pt[:, 64:128])

            def u_cc(tt, cc):
                # one contraction chunk of the K/Q/V0/V1 accumulations
                if cc == 0:
                    st8[tt] = (pmm.tile([128, TT], F32, tag="mm", name=f"psK{tt}"),
                               pmm.tile([128, TT], F32, tag="mm", name=f"psQ{tt}"),
                               pvt.tile([128, 128], F32, tag="pt", name=f"ptA{tt}"),
                               pvt.tile([128, 128], F32, tag="pt", name=f"ptB{tt}"))
                psK, psQ, ptA, ptB = st8[tt]
                st, sp = cc == 0, cc == NCC - 1
                nc.tensor.matmul(psQ[:], wqk[cc][:, 0:128], xts[cc][tt],
                                 start=st, stop=sp)
                nc.tensor.matmul(psK[:], wqk[cc][:, 128:256], xts[cc][tt],
                                 start=st, stop=sp)
                nc.tensor.matmul(ptA[:], xts[cc][tt][:, 0:128], wv[cc],
                                 start=st, stop=sp)
                nc.tensor.matmul(ptB[:], xts[cc][tt][:, 128:256], wv[cc],
                                 start=st, stop=sp)

            def u_fin(tt):
                psK, psQ, ptA, ptB = st8[tt]
                kT[tt] = cp.tile([128, TT], BF16, tag=f"k{tt}", name=f"k{tt}")
                nc.scalar.activation(
                    kT[tt][:], psK[:], mybir.ActivationFunctionType.Copy)
                qT[tt] = cp.tile([128, TT], BF16, tag=f"q{tt}", name=f"q{tt}")
                nc.vector.tensor_scalar_add(qT[tt][:], psQ[:], bq_sb[:, 0:1])
                vcopy(tt * 4 + 0, ptA)
                vcopy(tt * 4 + 1, ptB)

            def u_v23(tt, j):  # j in (2, 3)
                pt = pvt.tile([128, 128], F32, tag="pt")
                for cc in range(NCC):
                    nc.tensor.matmul(pt[:], xts[cc][tt][:, j * 128:(j + 1) * 128],
                                     wv[cc], start=(cc == 0), stop=(cc == NCC - 1))
                vcopy(tt * 4 + j, pt)

            def chain_rb(po, w):
                # softmax denominator: recip (DVE) then broadcast (Pool)
                rb = rbp.tile([1, w], BF16, tag=f"rb{w}")
                with nc.allow_low_precision(
                        reason="bf16 1/s is a uniform 0.4% softmax-scale "
                               "wobble, within the 2e-2 gate"):
                    nc.vector.reciprocal(rb[0:1, :], po[64:65, 0:w])
                rbc = rbp.tile([64, w], BF16, tag=f"rbc{w}")
                nc.gpsimd.partition_broadcast(rbc[:], rb[0:1, :])
                return rbc

            def chain_mul(po, rbc, hi, qt, c0, w):
                nc.vector.tensor_mul(yT[qt][hi * 64:(hi + 1) * 64, c0:c0 + w],
                                     po[0:64, 0:w], rbc[:])

            def emit_attn(qt, splits, fillers, borrow=None):
                """Causal attention for query tile qt. `splits` is a list of
                (c0, w, last_i): query column ranges with separate PSUM
                accumulators, each finishing at chunk index last_i —
                causality lets early splits finish before the last key
                chunks. The softmax chains are software-pipelined: split
                s+1's reciprocal is emitted before split s's multiply so the
                DVE queue overlaps the Pool broadcasts. Each diagonal chunk
                kcr processes only its live query range [kcr*128, 512) — a
                shifted tile whose first 128 columns hold the triangular
                diagonal block."""
                fit = iter(fillers)
                plan = ([qt * 4, qt * 4 + 1] + list(range(0, qt * 4))
                        + [qt * 4 + 2, qt * 4 + 3])
                if yT[qt] is None:
                    yT[qt] = cp.tile([128, TT], BF16, tag=f"y{qt}",
                                     name=f"y{qt}")
                for hi in range(2):
                    pos = []
                    for c0, w, _, pool in splits:
                        t = pool.tile([65 if pool is pov else 128, TT], F32,
                                      tag="po" if pool is pov else "pt",
                                      name=f"po{hi}_{qt}_{c0}")
                        pos.append(t[0:65, 0:w])
                    rbcs = [None] * len(splits)
                    muled = [False] * len(splits)

                    def pump(i):
                        # pipelined chain pump at end of chunk i: first emit
                        # recip+bcast for any split that just finished, then
                        # the muls whose rbc was emitted a chunk earlier
                        for s, (c0, w, last, _) in enumerate(splits):
                            if last == i:
                                rbcs[s] = chain_rb(pos[s], w)
                        for s, (c0, w, last, _) in enumerate(splits):
                            if rbcs[s] is not None and not muled[s] \
                                    and last < i:
                                chain_mul(pos[s], rbcs[s], hi, qt, c0, w)
                                muled[s] = True

                    for i, kc in enumerate(plan):
                        kcr = kc - qt * 4
                        shift = max(kcr, 0) * 128
                        width = TT - shift
                        ktile = kT[kc // 4]
                        kcol = (kc % 4) * 128
                        va = vaug[kc][:, hi * 65:(hi + 1) * 65]
                        # deepen the score ring with momentarily-idle banks
                        # from other pools (avoids the pst=2 ping-pong)
                        bp = borrow.get((hi, i)) if borrow else None
                        if bp is not None:
                            ps = bp.tile([128, TT], F32,
                                         tag="mm" if bp is pmm else "pt",
                                         name=f"sb{qt}_{hi}_{i}")
                        else:
                            ps = pst.tile([128, TT], F32, tag="st")
                        qs = qT[qt][hi * 64:(hi + 1) * 64, :]
                        nc.tensor.matmul(
                            ps[:, 0:width],
                            ktile[hi * 64:(hi + 1) * 64, kcol:kcol + 128],
                            qs[:, shift:TT], start=True, stop=True)
                        e = ep.tile([128, width], BF16, tag="e",
                                    name=f"e{qt}_{hi}_{i}")
                        nc.scalar.activation(
                            e[:], ps[:, 0:width],
                            mybir.ActivationFunctionType.Exp, scale=SCALE)
                        if kcr >= 0:  # diagonal block = first 128 columns
                            if kcr % 2 == 0:
                                nc.gpsimd.affine_select(
                                    e[:, 0:128], e[:, 0:128],
                                    pattern=[[1, 128]],
                                    compare_op=mybir.AluOpType.is_ge,
                                    fill=0.0, base=0, channel_multiplier=-1)
                            else:
                                nc.vector.tensor_mul(e[:, 0:128], e[:, 0:128],
                                                     mask0[:])
                        for s, (c0, w, last, _) in enumerate(splits):
                            lo = max(shift, c0)
                            hi_ = min(shift + width, c0 + w)
                            if hi_ > lo:
                                nc.tensor.matmul(
                                    pos[s][:, lo - c0:hi_ - c0], va,
                                    e[:, lo - shift:hi_ - shift],
                                    start=(i == 0), stop=(i == last))
                        pump(i)
                        f = next(fit, None)
                        if f is not None:
                            f()
                    for s, (c0, w, last, _) in enumerate(splits):
                        if not muled[s]:
                            chain_mul(pos[s], rbcs[s], hi, qt, c0, w)
                            muled[s] = True
                for f in fit:
                    if f is not None:
                        f()

            def u_et(tt, et, ost, col, eng="dve", pool=None):
                # full-width output-projection column group: single matmul,
                # contraction 128 = both heads' y rows stacked
                pool = pool or pmm
                tag = {id(pmm): "mm", id(pvt): "pt", id(pst): "st"}[id(pool)]
                pm = pool.tile([128, TT], F32, tag=tag)
                nc.tensor.matmul(pm[:], wpt[:, et * 128:(et + 1) * 128],
                                 yT[tt][:], start=True, stop=True)
                dst = ost[:, col * TT:(col + 1) * TT]
                if eng == "act":
                    nc.scalar.activation(
                        dst, pm[:], mybir.ActivationFunctionType.Copy)
                else:
                    nc.vector.tensor_copy(dst, pm[:])

            # --- un-fused A-half proj for tt1: hi0 prelaunched into held
            # --- psum banks during hi1's chunks, hi1 accumulated at tail
            pre = {}

            def u_pre(et, pool):
                tag = "mm" if pool is pmm else "pt"
                pre[et] = pool.tile([128, TT], F32, tag=tag,
                                    name=f"pre{et}")
                nc.tensor.matmul(
                    pre[et][:, 0:H], wpt[0:64, et * 128:(et + 1) * 128],
                    yT[1][0:64, 0:H], start=True, stop=False)

            def u_fina(et, ost, col, eng):
                nc.tensor.matmul(
                    pre[et][:, 0:H], wpt[64:128, et * 128:(et + 1) * 128],
                    yT[1][64:128, 0:H], start=False, stop=True)
                dst = ost[:, col * H:(col + 1) * H]
                if eng == "act":
                    nc.scalar.activation(dst, pre[et][:, 0:H],
                                         mybir.ActivationFunctionType.Copy)
                else:
                    nc.vector.tensor_copy(dst, pre[et][:, 0:H])

            def u_store(tt, et0, ngrp, ost):
                nc.sync.dma_start(
                    out=outT[et0 * 128:(et0 + ngrp) * 128,
                             tt * TT:(tt + 1) * TT]
                    .rearrange("(g p) t -> p g t", p=128),
                    in_=ost[:].rearrange("p (g t) -> p g t", g=ngrp))

            def proj_units(tt, groups):
                units = []
                for et0, ngrp in groups:
                    ost = cp.tile([128, ngrp * TT], BF16,
                                  tag=f"ost{tt}_{et0}", name=f"ost{tt}_{et0}")
                    for i in range(ngrp):
                        units.append(lambda tt=tt, et=et0 + i, ost=ost, col=i:
                                     u_et(tt, et, ost, col))
                    units.append(lambda tt=tt, et0=et0, ngrp=ngrp, ost=ost:
                                 u_store(tt, et0, ngrp, ost))
                return units

            # --- fused half-width proj for tt1 (half = 0 for A, 1 for B) ---
            def u_eth(et, half, ost, col, eng="dve", pool=None):
                pool = pool or pmm
                tag = {id(pmm): "mm", id(pvt): "pt", id(pst): "st"}[id(pool)]
                pm = pool.tile([128, TT], F32, tag=tag,
                               name=f"pm1_{et}_{half}")
                nc.tensor.matmul(
                    pm[:, 0:H], wpt[:, et * 128:(et + 1) * 128],
                    yT[1][:, half * H:(half + 1) * H], start=True, stop=True)
                dst = ost[:, col * H:(col + 1) * H]
                if eng == "act":
                    nc.scalar.activation(dst, pm[:, 0:H],
                                         mybir.ActivationFunctionType.Copy)
                else:
                    nc.vector.tensor_copy(dst, pm[:, 0:H])

            def u_store_h(half, et0, ngrp, ost):
                c0 = TT + half * H
                nc.sync.dma_start(
                    out=outT[et0 * 128:(et0 + ngrp) * 128, c0:c0 + H]
                    .rearrange("(g p) t -> p g t", p=128),
                    in_=ost[:].rearrange("p (g t) -> p g t", g=ngrp))

            # ---- phase C: qkv+vaug for tt0, DMA-paced per-cc ----
            for cc in range(NCC):
                u_cc(0, cc)
            u_fin(0)
            u_v23(0, 2)
            u_v23(0, 3)
            # ---- phase D: attn(qt0) interleaved with qkv+vaug(tt1) ----
            emit_attn(0, [(0, H, 1, pov), (H, H, 3, pov)],
                      [lambda cc=cc: u_cc(1, cc) for cc in range(NCC)]
                      + [lambda: u_fin(1), lambda: u_v23(1, 2)])
            # ---- phase E: attn(qt1); hi0 slots run proj(tt0); hi1 slots
            # ---- prelaunch the hi0 half of proj(tt1)-A into held banks ----
            pu0 = [lambda: u_v23(1, 3)]
            ost0a = cp.tile([128, 3 * TT], BF16, tag="ost0a")
            ost0b = cp.tile([128, 3 * TT], BF16, tag="ost0b")
            for i, et in enumerate(range(0, 3)):
                pu0.append(lambda et=et, i=i: u_et(
                    0, et, ost0a, i, eng=("act" if i % 2 else "dve")))
            pu0.append(lambda: u_store(0, 0, 3, ost0a))
            pu0.append(lambda: u_et(0, 3, ost0b, 0, eng="act"))
            pu0.append(lambda: u_pre(0, pmm))
            pu0.append(lambda: u_pre(1, pmm))

            ostA = cp.tile([128, 6 * H], BF16, tag="ost1A")
            ostB2 = cp.tile([128, 6 * H], BF16, tag="ost1B2")

            def post_e():
                # deferred tt0-proj columns: fill the A-chain cascade window
                # right after hi1's last AV (gated only by yT[0])
                u_et(0, 4, ost0b, 1, eng="dve", pool=pst)
                u_et(0, 5, ost0b, 2, eng="act", pool=pst)
                u_store(0, 3, 3, ost0b)

            def tail_a():
                # proj of tt1 half A: finish the prelaunched ets, fuse the
                # rest; runs while B's last chunks + B-chains go
                for col, et in enumerate(range(0, 2)):
                    u_fina(et, ostA, col, eng=("act" if col % 2 else "dve"))
                u_eth(2, 0, ostA, 2, eng="dve", pool=pmm)
                u_eth(3, 0, ostA, 3, eng="act", pool=pmm)
                u_eth(4, 0, ostA, 4, eng="dve", pool=pmm)
                u_eth(5, 0, ostA, 5, eng="act", pool=pmm)
                u_store_h(0, 0, 6, ostA)

            emit_attn(1, [(0, H, 5, pov), (H, 128, 6, pov),
                          (H + 128, 128, 7, pvt)],
                      pu0 + [
                None, None, None, None, None, None, None,
                lambda: (post_e(), tail_a())],
                borrow={(0, 3): pvt, (0, 6): pvt, (1, 2): pvt, (1, 5): pvt})
            # ---- phase F: proj of tt1 half B (B0 quarter matmuls fill the
            # ---- B1-chain window; B0/B1 share pm tiles so one copy each) ----
            pmq = {}
            for et, pool in [(0, pst), (1, pst), (2, pvt), (3, pvt),
                             (4, pmm), (5, pmm)]:
                tag = {id(pmm): "mm", id(pvt): "pt", id(pst): "st"}[id(pool)]
                pmq[et] = pool.tile([128, TT], F32, tag=tag, name=f"pmq{et}")
                nc.tensor.matmul(
                    pmq[et][:, 0:128], wpt[:, et * 128:(et + 1) * 128],
                    yT[1][:, H:H + 128], start=True, stop=True)
            for et in range(6):
                nc.tensor.matmul(
                    pmq[et][:, 128:256], wpt[:, et * 128:(et + 1) * 128],
                    yT[1][:, H + 128:TT], start=True, stop=True)
                dst = ostB2[:, et * H:(et + 1) * H]
                if et % 2:
                    nc.scalar.activation(dst, pmq[et][:, 0:H],
                                         mybir.ActivationFunctionType.Copy)
                else:
                    nc.vector.tensor_copy(dst, pmq[et][:, 0:H])
                if et == 3:
                    u_store_h(1, 0, 4, ostB2[:, 0:4 * H])
            u_store_h(1, 4, 2, ostB2[:, 4 * H:6 * H])
    nc.compile()
    return nc


def _in_maps(x, W_attn, b_attn, W_proj, b_proj):
    bf = ml_dtypes.bfloat16
    xTn = x.reshape(T, C).T.astype(np.float32)  # [C, T]
    maps = []
    for core in range(NCORES):
        h0, h1 = HEAD_MAP[core]
        cols = []
        for part in range(3):  # q, k, v column groups of W_attn
            for h in (h0, h1):
                cols.extend(range(part * C + h * HS, part * C + (h + 1) * HS))
        wsel = W_attn[:, cols].astype(np.float32)                       # [C, 384]
        bqc = np.concatenate([b_attn[h0 * HS:(h0 + 1) * HS],
                              b_attn[h1 * HS:(h1 + 1) * HS]]
                             ).astype(np.float32).reshape(128, 1)
        wpc = np.concatenate(
            [W_proj[h0 * HS:(h0 + 1) * HS, :],
             np.zeros_like(W_proj[:HS]) if h1 == h0
             else W_proj[h1 * HS:(h1 + 1) * HS, :]], axis=0)            # [128, C]
        blob = np.empty((128, BLOBW), np.float32)
        for cc in range(NCC):
            r = slice(cc * 128, (cc + 1) * 128)
            seg = cc * SEG
            blob[:, seg:seg + 256] = wsel[r, 0:256]
            blob[:, seg + 256:seg + 384] = wsel[r, 256:384]
            blob[:, seg + 384:seg + 896] = xTn[r, 0:TT]
            blob[:, X1OFF + cc * TT:X1OFF + (cc + 1) * TT] = xTn[r, TT:T]
        blob[:, WPOFF:BLOBW] = wpc
        maps.append({
            "blob": np.ascontiguousarray(blob.astype(bf)),
            "bq": np.ascontiguousarray(bqc),
        })
    return maps


def kernel(x, W_attn, b_attn, W_proj, b_proj, _trace=False, _trace_kwargs=None):
    x = np.asarray(x, np.float32)
    W_attn = np.asarray(W_attn, np.float32)
    b_attn = np.asarray(b_attn, np.float32)
    W_proj = np.asarray(W_proj, np.float32)
    b_proj = np.asarray(b_proj, np.float32)

    if "nc" not in _CACHE:
        _CACHE["nc"] = _build_program()
    nc = _CACHE["nc"]

    maps = _in_maps(x, W_attn, b_attn, W_proj, b_proj)
    kw = {}
    if _trace:
        kw = dict(trace=True, **(_trace_kwargs or {}))
    br = run_bass_kernel_spmd(nc, maps, list(range(NCORES)), **kw)
    acc = np.zeros((C, T), np.float64)
    for core in range(NCORES):
        acc += br.results[core]["outT"].astype(np.float64)
    # host-side bias fold: b_v @ W_proj + b_proj (softmax rows sum to 1)
    bias = (b_attn[2 * C:].astype(np.float64) @ W_proj.astype(np.float64)
            + b_proj.astype(np.float64))
    out = np.ascontiguousarray((acc.T + bias[None, :]).astype(np.float32))
    out = out.reshape(1, T, C)
    _CACHE["last_results"] = br
    return out



# revision 22
# speedup vs baseline: 1.0959x; 1.0959x over previous
"""TRN2 Bass kernel for nn_CausalSelfAttention_4054449128214.

The reference returns out_s + stop_gradient(out_full - out_s), whose forward
value is exactly out_full — plain dense causal self-attention. So the kernel
computes: qkv = x@W_attn+b_attn, per-head causal softmax attention, y@W_proj+b_proj.

Sharding (8 cores, no collectives):
  Megatron head-parallel. Cores 0-3 own head pairs (0,1)..(6,7); cores 4-7 own
  heads 8..11 (run twice for SPMD shape-uniformity, second copy's W_proj rows
  zeroed). Each core computes its heads' Q/K columns, V^T directly via matmul
  (lhsT = x chunk, rhs = W_v), attention, and a partial row-sliced output
  projection; the host sums the 8 partials (the Megatron row-parallel
  all-reduce) and transposes back.

Perf structure:
  - one consumption-ordered dram blob; per-cc segments (wqk|wv|x0) so each DMA
    unlocks a full contraction chunk of K/Q/V^T work (HWDGE issue is a serial
    625ns/DMA resource).
  - all matmul operands bf16 (full PE rate, half DMA bytes); fp32 PSUM.
  - PE p-state warmup: tiny matmuls during the initial DMA wait.
  - K/Q/V^T accumulation groups interleaved per-cc; attention interleaved
    with the next phase's matmuls.
  - attention query tiles split into 256-wide halves A/B with separate PSUM
    accumulators: causality lets half A skip the last two key chunks, so the
    A-half softmax/proj/store complete while B's chunks still run; the kernel
    tail is only the 256-wide B chain + 12 small proj matmuls + a tiny store.
  - bias algebra: b_k is softmax-invariant (dropped); b_v/b_proj fold into a
    host-side constant (softmax rows sum to 1); only b_q in-kernel.
"""

import numpy as np
import ml_dtypes

import concourse.bacc as bacc
import concourse.mybir as mybir
import concourse.tile as tile
from concourse.bass_utils import run_bass_kernel_spmd

F32 = mybir.dt.float32
BF16 = mybir.dt.bfloat16

T = 1024          # sequence length
C = 768           # channels
NH = 12           # heads
HS = 64           # head size
NCORES = 8
TT = 512          # t-tile (query tile)
H = TT // 2       # query half-tile
NT = T // TT      # 2
NCC = C // 128    # 6 contraction chunks
NKC = T // 128    # 8 key chunks
SCALE = 1.0 / 8.0  # 1/sqrt(HS)

NDUMMY = 200              # PE p-state warmup matmuls (run during initial DMA wait)
SEG = 896                 # per-cc blob segment: wqk(256) | wv(128) | x0(512)
X1OFF = NCC * SEG         # 5376
WPOFF = X1OFF + NCC * TT  # 8448
BLOBW = WPOFF + C         # 9216

# core -> (head0, head1); cores 4-7 duplicate their head (2nd W_proj slice zeroed)
HEAD_MAP = [(0, 1), (2, 3), (4, 5), (6, 7), (8, 8), (9, 9), (10, 10), (11, 11)]

_CACHE: dict = {}


def _build_program():
    nc = bacc.Bacc("TRN2", target_bir_lowering=False, debug=False,
                   num_devices=NCORES)
    blob = nc.dram_tensor("blob", [128, BLOBW], BF16, kind="ExternalInput").ap()
    bq = nc.dram_tensor("bq", [128, 1], F32, kind="ExternalInput").ap()
    outT = nc.dram_tensor("outT", [C, T], BF16, kind="ExternalOutput").ap()

    with tile.TileContext(nc) as tc:
        with (
            tc.tile_pool(name="const", bufs=1) as cp,
            tc.tile_pool(name="e", bufs=12) as ep,
            tc.tile_pool(name="rb", bufs=6) as rbp,
            tc.tile_pool(name="pmm", bufs=2, space="PSUM") as pmm,
            tc.tile_pool(name="pst", bufs=2, space="PSUM") as pst,
            tc.tile_pool(name="pov", bufs=2, space="PSUM") as pov,
            tc.tile_pool(name="pvt", bufs=2, space="PSUM") as pvt,
        ):
            bsb = cp.tile([128, WPOFF], BF16, tag="bsb")
            wpt = cp.tile([128, C], BF16, tag="wpt")
            for cc in range(NCC):
                nc.sync.dma_start(out=bsb[:, cc * SEG:(cc + 1) * SEG],
                                  in_=blob[:, cc * SEG:(cc + 1) * SEG])
            nc.sync.dma_start(out=bsb[:, X1OFF:X1OFF + 3 * TT],
                              in_=blob[:, X1OFF:X1OFF + 3 * TT])
            nc.sync.dma_start(out=bsb[:, X1OFF + 3 * TT:WPOFF],
                              in_=blob[:, X1OFF + 3 * TT:WPOFF])
            nc.sync.dma_start(out=wpt[:], in_=blob[:, WPOFF:BLOBW])
            bq_sb = cp.tile([128, 1], F32, tag="bq")
            nc.gpsimd.dma_start(out=bq_sb[:], in_=bq)

            wqk = [bsb[:, cc * SEG:cc * SEG + 256] for cc in range(NCC)]
            wv = [bsb[:, cc * SEG + 256:cc * SEG + 384] for cc in range(NCC)]
            xts = [[bsb[:, cc * SEG + 384:(cc + 1) * SEG],
                    bsb[:, X1OFF + cc * TT:X1OFF + (cc + 1) * TT]]
                   for cc in range(NCC)]

            ones64 = cp.tile([1, 64], BF16, tag="ones64")
            nc.vector.memset(ones64[:], 1.0)
            # PE warmup: tiny matmuls during the initial DMA wait keep the
            # tensor engine's p-state ramping so real work runs at full clock
            pdum = pvt.tile([1, 1], F32, tag="pt", name="pdum")
            for _ in range(NDUMMY):
                nc.tensor.matmul(pdum[:], ones64[0:1, 0:1], ones64[0:1, 0:1],
                                 start=True, stop=True)

            # V^T tiles: [128 keys, 65*2] with a ones column at 64 and 129
            vaug = [cp.tile([128, 130], BF16, tag=f"va{kc}", name=f"va{kc}")
                    for kc in range(NKC)]
            for kc in range(NKC):
                nc.vector.memset(vaug[kc][:, 64:130:65], 1.0)

            # static causal mask (keep where query-in-block >= key-in-block)
            # for the DVE half of the diagonal-mask work; with shifted tiles
            # the diagonal is always the first 128 columns of the tile
            mask0 = cp.tile([128, 128], BF16, tag="mask0")
            nc.vector.memset(mask0[:], 1.0)
            nc.gpsimd.affine_select(
                mask0[:], mask0[:], pattern=[[1, 128]],
                compare_op=mybir.AluOpType.is_ge, fill=0.0,
                base=0, channel_multiplier=-1)

            qT = [None] * NT   # [128=(2h x 64hs), TT] bf16
            kT = [None] * NT
            yT = [None] * NT   # [128=(2h x 64hs), TT] bf16 (both heads stacked)
            st8 = {}           # per-tt K/Q/V01 psum state

            def vcopy(kc, pt):
                nc.vector.tensor_copy(vaug[kc][:, 0:64], pt[:, 0:64])
                nc.vector.tensor_copy(vaug[kc][:, 65:129], pt[:, 64:128])

            def u_cc(tt, cc):
                # one contraction chunk of the K/Q/V0/V1 accumulations
                if cc == 0:
                    st8[tt] = (pmm.tile([128, TT], F32, tag="mm", name=f"psK{tt}"),
                               pmm.tile([128, TT], F32, tag="mm", name=f"psQ{tt}"),
                               pvt.tile([128, 128], F32, tag="pt", name=f"ptA{tt}"),
                               pvt.tile([128, 128], F32, tag="pt", name=f"ptB{tt}"))
                psK, psQ, ptA, ptB = st8[tt]
                st, sp = cc == 0, cc == NCC - 1
                nc.tensor.matmul(psQ[:], wqk[cc][:, 0:128], xts[cc][tt],
                                 start=st, stop=sp)
                nc.tensor.matmul(psK[:], wqk[cc][:, 128:256], xts[cc][tt],
                                 start=st, stop=sp)
                nc.tensor.matmul(ptA[:], xts[cc][tt][:, 0:128], wv[cc],
                                 start=st, stop=sp)
                nc.tensor.matmul(ptB[:], xts[cc][tt][:, 128:256], wv[cc],
                                 start=st, stop=sp)

            def u_fin(tt):
                psK, psQ, ptA, ptB = st8[tt]
                kT[tt] = cp.tile([128, TT], BF16, tag=f"k{tt}", name=f"k{tt}")
                nc.scalar.activation(
                    kT[tt][:], psK[:], mybir.ActivationFunctionType.Copy)
                qT[tt] = cp.tile([128, TT], BF16, tag=f"q{tt}", name=f"q{tt}")
                nc.vector.tensor_scalar_add(qT[tt][:], psQ[:], bq_sb[:, 0:1])
                vcopy(tt * 4 + 0, ptA)
                vcopy(tt * 4 + 1, ptB)

            def u_v23(tt, j):  # j in (2, 3)
                pt = pvt.tile([128, 128], F32, tag="pt")
                for cc in range(NCC):
                    nc.tensor.matmul(pt[:], xts[cc][tt][:, j * 128:(j + 1) * 128],
                                     wv[cc], start=(cc == 0), stop=(cc == NCC - 1))
                vcopy(tt * 4 + j, pt)

            def chain_rb(po, w):
                # softmax denominator: recip (DVE) then broadcast (Pool)
                rb = rbp.tile([1, w], BF16, tag=f"rb{w}")
                with nc.allow_low_precision(
                        reason="bf16 1/s is a uniform 0.4% softmax-scale "
                               "wobble, within the 2e-2 gate"):
                    nc.vector.reciprocal(rb[0:1, :], po[64:65, 0:w])
                rbc = rbp.tile([64, w], BF16, tag=f"rbc{w}")
                nc.gpsimd.partition_broadcast(rbc[:], rb[0:1, :])
                return rbc

            def chain_mul(po, rbc, hi, qt, c0, w):
                nc.vector.tensor_mul(yT[qt][hi * 64:(hi + 1) * 64, c0:c0 + w],
                                     po[0:64, 0:w], rbc[:])

            def emit_attn(qt, splits, fillers, borrow=None):
                """Causal attention for query tile qt. `splits` is a list of
                (c0, w, last_i): query column ranges with separate PSUM
                accumulators, each finishing at chunk index last_i —
                causality lets early splits finish before the last key
                chunks. The softmax chains are software-pipelined: split
                s+1's reciprocal is emitted before split s's multiply so the
                DVE queue overlaps the Pool broadcasts. Each diagonal chunk
                kcr processes only its live query range [kcr*128, 512) — a
                shifted tile whose first 128 columns hold the triangular
                diagonal block."""
                fit = iter(fillers)
                plan = ([qt * 4, qt * 4 + 1] + list(range(0, qt * 4))
                        + [qt * 4 + 2, qt * 4 + 3])
                if yT[qt] is None:
                    yT[qt] = cp.tile([128, TT], BF16, tag=f"y{qt}",
                                     name=f"y{qt}")
                for hi in range(2):
                    pos = []
                    for c0, w, _, pool in splits:
                        t = pool.tile([65 if pool is pov else 128, TT], F32,
                                      tag="po" if pool is pov else "pt",
                                      name=f"po{hi}_{qt}_{c0}")
                        pos.append(t[0:65, 0:w])
                    rbcs = [None] * len(splits)
                    muled = [False] * len(splits)

                    def pump(i):
                        # chain pump at end of chunk i: recip+bcast for any
                        # split that just finished (plus its mul immediately
                        # for the final split, so no filler work can get
                        # ahead of it in the DVE queue), then the muls whose
                        # rbc was emitted a chunk earlier
                        for s, (c0, w, last, _) in enumerate(splits):
                            if last == i:
                                rbcs[s] = chain_rb(pos[s], w)
                                if s == len(splits) - 1:
                                    chain_mul(pos[s], rbcs[s], hi, qt, c0, w)
                                    muled[s] = True
                        for s, (c0, w, last, _) in enumerate(splits):
                            if rbcs[s] is not None and not muled[s] \
                                    and last < i:
                                chain_mul(pos[s], rbcs[s], hi, qt, c0, w)
                                muled[s] = True

                    for i, kc in enumerate(plan):
                        kcr = kc - qt * 4
                        shift = max(kcr, 0) * 128
                        width = TT - shift
                        ktile = kT[kc // 4]
                        kcol = (kc % 4) * 128
                        va = vaug[kc][:, hi * 65:(hi + 1) * 65]
                        # deepen the score ring with momentarily-idle banks
                        # from other pools (avoids the pst=2 ping-pong)
                        bp = borrow.get((hi, i)) if borrow else None
                        if bp is not None:
                            ps = bp.tile([128, TT], F32,
                                         tag="mm" if bp is pmm else "pt",
                                         name=f"sb{qt}_{hi}_{i}")
                        else:
                            ps = pst.tile([128, TT], F32, tag="st")
                        qs = qT[qt][hi * 64:(hi + 1) * 64, :]
                        nc.tensor.matmul(
                            ps[:, 0:width],
                            ktile[hi * 64:(hi + 1) * 64, kcol:kcol + 128],
                            qs[:, shift:TT], start=True, stop=True)
                        e = ep.tile([128, width], BF16, tag="e",
                                    name=f"e{qt}_{hi}_{i}")
                        nc.scalar.activation(
                            e[:], ps[:, 0:width],
                            mybir.ActivationFunctionType.Exp, scale=SCALE)
                        if kcr >= 0:  # diagonal block = first 128 columns
                            if kcr % 2 == 0:
                                nc.gpsimd.affine_select(
                                    e[:, 0:128], e[:, 0:128],
                                    pattern=[[1, 128]],
                                    compare_op=mybir.AluOpType.is_ge,
                                    fill=0.0, base=0, channel_multiplier=-1)
                            else:
                                nc.vector.tensor_mul(e[:, 0:128], e[:, 0:128],
                                                     mask0[:])
                        for s, (c0, w, last, _) in enumerate(splits):
                            lo = max(shift, c0)
                            hi_ = min(shift + width, c0 + w)
                            if hi_ > lo:
                                nc.tensor.matmul(
                                    pos[s][:, lo - c0:hi_ - c0], va,
                                    e[:, lo - shift:hi_ - shift],
                                    start=(i == 0), stop=(i == last))
                        pump(i)
                        f = next(fit, None)
                        if f is not None:
                            f()
                    for s, (c0, w, last, _) in enumerate(splits):
                        if not muled[s]:
                            chain_mul(pos[s], rbcs[s], hi, qt, c0, w)
                            muled[s] = True
                for f in fit:
                    if f is not None:
                        f()

            def u_et(tt, et, ost, col, eng="dve", pool=None):
                # full-width output-projection column group: single matmul,
                # contraction 128 = both heads' y rows stacked
                pool = pool or pmm
                tag = {id(pmm): "mm", id(pvt): "pt", id(pst): "st"}[id(pool)]
                pm = pool.tile([128, TT], F32, tag=tag)
                nc.tensor.matmul(pm[:], wpt[:, et * 128:(et + 1) * 128],
                                 yT[tt][:], start=True, stop=True)
                dst = ost[:, col * TT:(col + 1) * TT]
                if eng == "act":
                    nc.scalar.activation(
                        dst, pm[:], mybir.ActivationFunctionType.Copy)
                else:
                    nc.vector.tensor_copy(dst, pm[:])

            # --- un-fused A-half proj for tt1: hi0 prelaunched into held
            # --- psum banks during hi1's chunks, hi1 accumulated at tail
            pre = {}

            def u_pre(et, pool):
                tag = "mm" if pool is pmm else "pt"
                pre[et] = pool.tile([128, TT], F32, tag=tag,
                                    name=f"pre{et}")
                nc.tensor.matmul(
                    pre[et][:, 0:H], wpt[0:64, et * 128:(et + 1) * 128],
                    yT[1][0:64, 0:H], start=True, stop=False)

            def u_fina(et, ost, col, eng):
                nc.tensor.matmul(
                    pre[et][:, 0:H], wpt[64:128, et * 128:(et + 1) * 128],
                    yT[1][64:128, 0:H], start=False, stop=True)
                dst = ost[:, col * H:(col + 1) * H]
                if eng == "act":
                    nc.scalar.activation(dst, pre[et][:, 0:H],
                                         mybir.ActivationFunctionType.Copy)
                else:
                    nc.vector.tensor_copy(dst, pre[et][:, 0:H])

            def u_store(tt, et0, ngrp, ost):
                nc.sync.dma_start(
                    out=outT[et0 * 128:(et0 + ngrp) * 128,
                             tt * TT:(tt + 1) * TT]
                    .rearrange("(g p) t -> p g t", p=128),
                    in_=ost[:].rearrange("p (g t) -> p g t", g=ngrp))

            def proj_units(tt, groups):
                units = []
                for et0, ngrp in groups:
                    ost = cp.tile([128, ngrp * TT], BF16,
                                  tag=f"ost{tt}_{et0}", name=f"ost{tt}_{et0}")
                    for i in range(ngrp):
                        units.append(lambda tt=tt, et=et0 + i, ost=ost, col=i:
                                     u_et(tt, et, ost, col))
                    units.append(lambda tt=tt, et0=et0, ngrp=ngrp, ost=ost:
                                 u_store(tt, et0, ngrp, ost))
                return units

            # --- fused half-width proj for tt1 (half = 0 for A, 1 for B) ---
            def u_eth(et, half, ost, col, eng="dve", pool=None):
                pool = pool or pmm
                tag = {id(pmm): "mm", id(pvt): "pt", id(pst): "st"}[id(pool)]
                pm = pool.tile([128, TT], F32, tag=tag,
                               name=f"pm1_{et}_{half}")
                nc.tensor.matmul(
                    pm[:, 0:H], wpt[:, et * 128:(et + 1) * 128],
                    yT[1][:, half * H:(half + 1) * H], start=True, stop=True)
                dst = ost[:, col * H:(col + 1) * H]
                if eng == "act":
                    nc.scalar.activation(dst, pm[:, 0:H],
                                         mybir.ActivationFunctionType.Copy)
                else:
                    nc.vector.tensor_copy(dst, pm[:, 0:H])

            def u_store_h(half, et0, ngrp, ost):
                c0 = TT + half * H
                nc.sync.dma_start(
                    out=outT[et0 * 128:(et0 + ngrp) * 128, c0:c0 + H]
                    .rearrange("(g p) t -> p g t", p=128),
                    in_=ost[:].rearrange("p (g t) -> p g t", g=ngrp))

            # ---- phase C: qkv+vaug for tt0, DMA-paced per-cc ----
            for cc in range(NCC):
                u_cc(0, cc)
            u_fin(0)
            u_v23(0, 2)
            u_v23(0, 3)
            # ---- phase D: attn(qt0) interleaved with qkv+vaug(tt1) ----
            emit_attn(0, [(0, H, 1, pov), (H, H, 3, pov)],
                      [lambda cc=cc: u_cc(1, cc) for cc in range(NCC)]
                      + [lambda: u_fin(1), lambda: u_v23(1, 2)])
            # ---- phase E: attn(qt1); hi0 slots run proj(tt0); hi1 slots
            # ---- run deferred proj(tt0) + prelaunch hi0 of proj(tt1)-A ----
            ost0a = cp.tile([128, 3 * TT], BF16, tag="ost0a")
            ost0b = cp.tile([128, 3 * TT], BF16, tag="ost0b")
            pu0 = [lambda: u_v23(1, 3)]
            for i, et in enumerate(range(0, 3)):
                pu0.append(lambda et=et, i=i: u_et(
                    0, et, ost0a, i, eng=("act" if i % 2 else "dve")))
            pu0.append(lambda: u_store(0, 0, 3, ost0a))
            for i, et in enumerate(range(3, 6)):
                pu0.append(lambda et=et, i=i: u_et(
                    0, et, ost0b, i, eng=("dve" if i % 2 else "act")))

            ostA = cp.tile([128, 6 * H], BF16, tag="ost1A")
            ostB2 = cp.tile([128, 6 * H], BF16, tag="ost1B2")

            def tail_a():
                # proj of tt1 half A: finish the prelaunched ets, fuse the
                # rest; runs while the B-chain cascade goes
                for col, et in enumerate(range(0, 4)):
                    u_fina(et, ostA, col, eng=("act" if col % 2 else "dve"))
                u_eth(4, 0, ostA, 4, eng="dve", pool=pmm)
                u_eth(5, 0, ostA, 5, eng="act", pool=pmm)
                u_store_h(0, 0, 6, ostA)

            emit_attn(1, [(0, H, 5, pov), (H, H, 7, pov)],
                      pu0 + [
                lambda: u_store(0, 3, 3, ost0b),
                lambda: u_pre(0, pmm), lambda: u_pre(1, pmm),
                lambda: u_pre(2, pvt), lambda: u_pre(3, pvt),
                None, None, None, tail_a],
                borrow={(0, 5): pvt, (1, 2): pvt})
            # ---- phase F: proj of tt1 half B (the only true tail) ----
            for col, (et, pool, eng) in enumerate(
                    [(0, pst, "dve"), (1, pst, "act"), (2, pvt, "dve"),
                     (3, pvt, "act")]):
                u_eth(et, 1, ostB2, col, eng=eng, pool=pool)
            u_store_h(1, 0, 4, ostB2[:, 0:4 * H])
            u_eth(4, 1, ostB2, 4, eng="dve", pool=pmm)
            u_eth(5, 1, ostB2, 5, eng="act", pool=pmm)
            u_store_h(1, 4, 2, ostB2[:, 4 * H:6 * H])
    nc.compile()
    return nc


def _in_maps(x, W_attn, b_attn, W_proj, b_proj):
    bf = ml_dtypes.bfloat16
    xTn = x.reshape(T, C).T.astype(np.float32)  # [C, T]
    maps = []
    for core in range(NCORES):
        h0, h1 = HEAD_MAP[core]
        cols = []
        for part in range(3):  # q, k, v column groups of W_attn
            for h in (h0, h1):
                cols.extend(range(part * C + h * HS, part * C + (h + 1) * HS))
        wsel = W_attn[:, cols].astype(np.float32)                       # [C, 384]
        bqc = np.concatenate([b_attn[h0 * HS:(h0 + 1) * HS],
                              b_attn[h1 * HS:(h1 + 1) * HS]]
                             ).astype(np.float32).reshape(128, 1)
        wpc = np.concatenate(
            [W_proj[h0 * HS:(h0 + 1) * HS, :],
             np.zeros_like(W_proj[:HS]) if h1 == h0
             else W_proj[h1 * HS:(h1 + 1) * HS, :]], axis=0)            # [128, C]
        blob = np.empty((128, BLOBW), np.float32)
        for cc in range(NCC):
            r = slice(cc * 128, (cc + 1) * 128)
            seg = cc * SEG
            blob[:, seg:seg + 256] = wsel[r, 0:256]
            blob[:, seg + 256:seg + 384] = wsel[r, 256:384]
            blob[:, seg + 384:seg + 896] = xTn[r, 0:TT]
            blob[:, X1OFF + cc * TT:X1OFF + (cc + 1) * TT] = xTn[r, TT:T]
        blob[:, WPOFF:BLOBW] = wpc
        maps.append({
            "blob": np.ascontiguousarray(blob.astype(bf)),
            "bq": np.ascontiguousarray(bqc),
        })
    return maps


def kernel(x, W_attn, b_attn, W_proj, b_proj, _trace=False, _trace_kwargs=None):
    x = np.asarray(x, np.float32)
    W_attn = np.asarray(W_attn, np.float32)
    b_attn = np.asarray(b_attn, np.float32)
    W_proj = np.asarray(W_proj, np.float32)
    b_proj = np.asarray(b_proj, np.float32)

    if "nc" not in _CACHE:
        _CACHE["nc"] = _build_program()
    nc = _CACHE["nc"]

    maps = _in_maps(x, W_attn, b_attn, W_proj, b_proj)
    kw = {}
    if _trace:
        kw = dict(trace=True, **(_trace_kwargs or {}))
    br = run_bass_kernel_spmd(nc, maps, list(range(NCORES)), **kw)
    acc = np.zeros((C, T), np.float64)
    for core in range(NCORES):
        acc += br.results[core]["outT"].astype(np.float64)
    # host-side bias fold: b_v @ W_proj + b_proj (softmax rows sum to 1)
    bias = (b_attn[2 * C:].astype(np.float64) @ W_proj.astype(np.float64)
            + b_proj.astype(np.float64))
    out = np.ascontiguousarray((acc.T + bias[None, :]).astype(np.float32))
    out = out.reshape(1, T, C)
    _CACHE["last_results"] = br
    return out

